# revision 26
# baseline (speedup 1.0000x reference)
"""Trainium2 Bass kernel for Conv2Demod (StyleGAN2-style modulated conv),
Winograd F(2x2, 3x3) formulation.

Reference computation (per sample b):
    w[b,o,i,ky,kx] = weight[o,i,ky,kx] * (1 + s[b,i])
    d[b,o]         = rsqrt(sum_{i,ky,kx} w^2 + 1e-8)
    out[b]         = conv2d(img[b], w[b]*d[b,o], pad=1)

Algebraic restructuring:
  - The per-input-channel (1+s_i) modulation commutes through the conv's
    channel contraction AND the (linear) Winograd transforms, so it is
    applied once to the image (per-partition activation scale).
  - Winograd F(2,3): out tiles of 2x2 from input patches of 4x4 (stride 2).
    G w G^T is host-precomputed (static); B^T d B runs on GpSimd+DVE as
    separable +/- row/col passes; the 16 per-position GEMMs contract
    cin=512 on the PE (2.25x fewer streamed columns than direct conv);
    A^T m A runs on DVE while evicting.
  - The demodulation d[b,o] is applied as a per-partition ACT scale while
    evicting GEMM results from PSUM.
  - sum w^2 = (sum_k weight^2)^T @ (1+s)^2 -- a tiny on-device matvec from
    the host-precomputed static A_T[i,o] = sum_k weight[o,i,:,:]^2.

Sharding: data-parallel over batch -- 8 samples onto 8 NeuronCores, one
sample per core; weight inputs replicated.

Pipeline: tiles (ty,tx in 0..31) processed in 4 chunks of 8 ty-rows (256
tiles, GEMM N=256 = half a PSUM bank; 16 uv positions fit 8 banks in
pairs). Input-transform production for chunk c+1 is EMITTED before the
inverse-transform of chunk c so the DVE never stalls behind the PE.
All conv arithmetic in bf16 (FWL weight loads), PSUM accumulation fp32;
end-to-end rel err ~6e-3 (validated vs reference).
"""

import contextlib

import numpy as np

import concourse.bacc as bacc
import concourse.mybir as mybir
import concourse.tile as tile
from concourse.bass_utils import run_bass_kernel_spmd

P = 128          # partitions
CIN = 512
COUT = 512
H = W = 64
NI = CIN // P    # 4 i-tiles
NO = COUT // P   # 4 o-tiles
HP = WP = H + 2  # padded
EPS = 1e-8
N_CORES = 8

NUV = 16         # Winograd positions (4x4)
TY = TX = 32     # output tile grid (2x2 out per tile)
NCHUNK = 4       # ty-chunks
TYC = TY // NCHUNK            # 8 ty rows per chunk
NT = TYC * TX                 # 256 tiles per chunk = GEMM N
ROWS = 2 * TYC + 2            # 18 padded image rows per chunk

# B^T row u reads patch rows (a1 op a2); same pattern for columns (v)
BCOMB = [(0, "sub", 2), (1, "add", 2), (2, "sub", 1), (1, "sub", 3)]

F32 = mybir.dt.float32
F32R = mybir.dt.float32r
BF16 = mybir.dt.bfloat16
AF = mybir.ActivationFunctionType
_nullcm = contextlib.nullcontext


def build_nc(chain=False, loop_n=None):
    nc = bacc.Bacc("TRN2", target_bir_lowering=False, debug=False)

    # host-padded bf16 image: [i_tile, partition, 66, 66] with zero border
    img = nc.dram_tensor("img", [NI, P, HP, WP], BF16, kind="ExternalInput").ap()
    s_in = nc.dram_tensor("s", [CIN], F32, kind="ExternalInput").ap()
    # host Winograd-transformed weights: [i_tile, partition, uv, cout] bf16
    wt = nc.dram_tensor("wt", [NI, P, NUV, COUT], BF16, kind="ExternalInput").ap()
    at = nc.dram_tensor("at", [CIN, COUT], BF16, kind="ExternalInput").ap()
    out = nc.dram_tensor("out", [COUT, H, W], F32, kind="ExternalOutput").ap()
    s_out = None
    if chain:
        s_out = nc.dram_tensor("s_out", [CIN], F32, kind="ExternalOutput").ap()

    with tile.TileContext(nc) as tc:
        with (
            tc.tile_pool(name="const", bufs=1) as cpool,
            tc.tile_pool(name="persist", bufs=1) as ppool,
            tc.tile_pool(name="rawp", bufs=2) as rawp,
            tc.tile_pool(name="tmpp", bufs=1) as tmpp,
            tc.tile_pool(name="inp", bufs=2) as inp,
            tc.tile_pool(name="mp", bufs=2) as mp,
            tc.tile_pool(name="qp", bufs=2) as qp,
            tc.tile_pool(name="op", bufs=2) as op,
            tc.tile_pool(name="psum", bufs=3, space="PSUM") as psum_pool,
            tc.tile_pool(name="psum_d", bufs=1, space="PSUM") as psum_d,
        ):
            with (tc.For_i(0, loop_n, 1) if loop_n else _nullcm()):
                # ---- s-derived scalars -------------------------------------
                sraw = cpool.tile([P, NI, 2], F32, tag="sraw")
                for c in range(2):
                    nc.sync.dma_start(
                        sraw[:, :, c], s_in.rearrange("(t p) -> p t", p=P)
                    )
                if chain:
                    nc.sync.dma_start(s_out[:], s_in[:])
                smod = cpool.tile([P, NI, 2], F32, tag="smod")  # 1 + s
                nc.scalar.activation(smod[:], sraw[:], AF.Copy, bias=1.0)
                tsq = cpool.tile([P, NI, 2], BF16, tag="tsq")   # (1 + s)^2
                nc.scalar.square(tsq[:], smod[:])

                # ---- demod d[o] = 1/sqrt(A_T.T @ tsq + eps) ----------------
                at_sb = ppool.tile([P, NI, COUT], BF16, tag="at_sb")
                nc.sync.dma_start(
                    at_sb[:], at.rearrange("(t p) o -> p t o", p=P)
                )
                dsb = cpool.tile([P, NO], F32, tag="dsb")
                dtmp = cpool.tile([P, NO], F32, tag="dtmp")
                epst = cpool.tile([P, 1], F32, tag="epst")
                nc.vector.memset(epst[:], EPS)
                for ot in range(NO):
                    o0 = ot * P
                    psd = psum_d.tile([P, 2], F32)
                    for it in range(NI):
                        nc.tensor.matmul(
                            psd[:],
                            at_sb[:, it, o0 : o0 + P],
                            tsq[:, it, :],
                            start=(it == 0),
                            stop=(it == NI - 1),
                        )
                    nc.scalar.activation(
                        dtmp[:, ot : ot + 1], psd[:, 0:1], AF.Sqrt, bias=epst[:]
                    )
                nc.vector.reciprocal(dsb[:], dtmp[:])

                # ---- Winograd weights straight to SBUF ---------------------
                wsb = []
                for it in range(NI):
                    t = ppool.tile([P, NUV, COUT], BF16, tag=f"wsb{it}")
                    wsb.append(t)
                    nc.sync.dma_start(t[:], wt[it])

                # ---- input transform: In[uv] = B^T d B  (per chunk) --------
                # The (1+s_i) modulation is an in-place per-partition DVE
                # scale on each fresh raw chunk: gated only by that chunk's
                # small DMA, so it never head-of-line-blocks the DVE queue
                # the way scaling the (bulk-reloaded) weights would.
                def emit_produce(ch):
                    r0 = ch * 2 * TYC
                    in_t = inp.tile([P, NI, NUV, NT], BF16)
                    raw = rawp.tile([P, NI, ROWS, WP], BF16)
                    for it in range(NI):
                        nc.sync.dma_start(
                            raw[:, it], img[it][:, r0 : r0 + ROWS, :]
                        )
                        nc.vector.tensor_scalar_mul(
                            raw[:, it], raw[:, it], smod[:, it, 0:1]
                        )
                    # row pass: tmp[it, u, ty, x], all 4 i-tiles per op
                    tmp = tmpp.tile([P, NI, 4, TYC, WP], BF16)
                    for u, (a1, alu, a2) in enumerate(BCOMB):
                        f = (
                            nc.vector.tensor_add
                            if alu == "add"
                            else nc.vector.tensor_sub
                        )
                        f(
                            tmp[:, :, u],
                            raw[:, :, a1 : a1 + 2 * TYC - 1 : 2, :],
                            raw[:, :, a2 : a2 + 2 * TYC - 1 : 2, :],
                        )
                    # col pass: In[4u+v, ty, tx]; the v-combination is
                    # identical for every u, so all 4 u at once
                    for it in range(NI):
                        for v, (b1, alu, b2) in enumerate(BCOMB):
                            f = (
                                nc.vector.tensor_add
                                if alu == "add"
                                else nc.vector.tensor_sub
                            )
                            f(
                                in_t[:, it, v : NUV : 4, :].rearrange(
                                    "p u (a b) -> p u a b", b=TX
                                ),
                                tmp[:, it, :, :, b1 : b1 + 2 * TX - 1 : 2],
                                tmp[:, it, :, :, b2 : b2 + 2 * TX - 1 : 2],
                            )
                    return in_t

                # ---- GEMMs + inverse transform (per chunk) -----------------
                def emit_consume(ch, in_t):
                    for ot in range(NO):
                        o0 = ot * P
                        m = mp.tile([P, NUV, NT], BF16)
                        for g in range(4):
                            # 4 uv accumulation chains share a 2-bank tile
                            ps = psum_pool.tile([P, 4, NT], F32)
                            for q4 in range(4):
                                uv = 4 * g + q4
                                for it in range(NI):
                                    nc.tensor.matmul(
                                        ps[:, q4],
                                        wsb[it][:, uv, o0 : o0 + P],
                                        in_t[:, it, uv, :],
                                        start=(it == 0),
                                        stop=(it == NI - 1),
                                        skip_group_check=True,
                                    )
                            # evict 4 uv at once, demod scale applied here
                            nc.scalar.activation(
                                m[:, 4 * g : 4 * g + 4, :], ps[:],
                                AF.Copy, scale=dsb[:, ot : ot + 1],
                            )
                        # inverse transform A^T m A (DVE), m indexed uv=4u+v;
                        # the u-combination is identical for every v, so all
                        # 4 v per op
                        q = qp.tile([P, 2, 4, NT], BF16)
                        nc.vector.tensor_add(q[:, 0], m[:, 0:4, :], m[:, 4:8, :])
                        nc.vector.tensor_add(q[:, 0], q[:, 0], m[:, 8:12, :])
                        nc.vector.tensor_sub(q[:, 1], m[:, 4:8, :], m[:, 8:12, :])
                        nc.vector.tensor_sub(q[:, 1], q[:, 1], m[:, 12:16, :])
                        # stage2: both dy at once via 4D strided views
                        o_t = op.tile([P, 2 * TYC, W], F32)
                        ov = o_t[:].rearrange(
                            "p (a d) (b e) -> p a d b e", d=2, e=2
                        )
                        qv = [
                            q[:, :, v, :].rearrange("p d (a b) -> p a d b", b=TX)
                            for v in range(4)
                        ]
                        o00, o01 = ov[:, :, :, :, 0], ov[:, :, :, :, 1]
                        nc.vector.tensor_add(o00, qv[0], qv[1])
                        nc.vector.tensor_add(o00, o00, qv[2])
                        nc.vector.tensor_sub(o01, qv[1], qv[2])
                        nc.vector.tensor_sub(o01, o01, qv[3])
                        nc.sync.dma_start(
                            out[o0 : o0 + P, ch * 2 * TYC : (ch + 1) * 2 * TYC, :],
                            o_t[:],
                        )

                in_tiles = {0: emit_produce(0)}
                for ch in range(NCHUNK):
                    if ch + 1 < NCHUNK:
                        in_tiles[ch + 1] = emit_produce(ch + 1)
                    emit_consume(ch, in_tiles.pop(ch))
    nc.compile()
    return nc


_NC_CACHE = None


def _get_nc():
    global _NC_CACHE
    if _NC_CACHE is None:
        _NC_CACHE = build_nc()
    return _NC_CACHE


_BT = np.array(
    [[1, 0, -1, 0], [0, 1, 1, 0], [0, -1, 1, 0], [0, 1, 0, -1]], np.float64
)
_G = np.array(
    [[1, 0, 0], [0.5, 0.5, 0.5], [0.5, -0.5, 0.5], [0, 0, 1]], np.float64
)


def make_in_maps(img, s, weight):
    """Host-side input prep: shard over batch, static weight transforms."""
    img = np.asarray(img, dtype=np.float32)
    s = np.ascontiguousarray(np.asarray(s, dtype=np.float32))
    weight = np.asarray(weight, dtype=np.float32)
    bf16 = mybir.dt.np(BF16)
    # zero-pad image host-side, cast bf16: [B, NI, P, HP, WP]
    imgp = np.zeros((img.shape[0], NI, P, HP, WP), dtype=np.float32)
    imgp[:, :, :, 1 : H + 1, 1 : W + 1] = img.reshape(-1, NI, P, H, W)
    imgp = imgp.astype(bf16)
    # Winograd weight transform G w G^T: [O,I,3,3] -> [NI, P, uv, O] bf16
    gw = np.einsum("ua,oiab,vb->iuvo", _G, weight.astype(np.float64), _G)
    wtw = np.ascontiguousarray(
        gw.reshape(CIN, NUV, COUT).reshape(NI, P, NUV, COUT).astype(bf16)
    )
    # A_T[i, o] = sum_k weight[o, i, :, :]^2  (static, sample-independent)
    at = np.ascontiguousarray(
        (weight.astype(np.float64) ** 2).sum(axis=(2, 3)).T.astype(bf16)
    )
    return [
        {"img": imgp[b], "s": s[b], "wt": wtw, "at": at} for b in range(N_CORES)
    ]


def kernel(img, s, weight):
    nc = _get_nc()
    in_maps = make_in_maps(img, s, weight)
    res = run_bass_kernel_spmd(nc, in_maps, list(range(N_CORES)))
    return np.stack([res.results[b]["out"] for b in range(N_CORES)], axis=0)
